# revision 1
# baseline (speedup 1.0000x reference)
"""Causal single-head attention block for Trainium2, SPMD across 8 NeuronCores.

Problem (hardcoded):
    x:     [4, 2048, 1024] f32
    w_qkv: [1024, 3072]    f32   (q | k | v column blocks)
    w_out: [1024, 1024]    f32
    b_out: [1024]          f32
    y = softmax(causal(q @ k.T / 32)) @ v @ w_out + b_out     -> [4, 2048, 1024]

Algebraic folding (host-side, fp32):
    sim  = (x wq)(x wk)^T = x (wq wk^T) x^T          -> Mq  = wq @ wk.T
    out  = attn (x wv) wo = attn x (wv wo)           -> Wvo = wv @ w_out
so the device kernel never materializes Q/K/V: it computes
    QM^T = Mq^T x_q^T   (local queries only)
    sim  = QM x^T       (x^T SBUF-resident)
    attnX = softmax(causal(sim)) @ x                 (x rows streamed)
    y    = attnX @ Wvo + b
This halves the tensor-engine work vs. projecting Q/K/V explicitly and
removes the duplicated K/V computation across the core pair.

Sharding: 2 cores per batch element. Within a batch, the 16 query subtiles of
128 rows are dealt round-robin to the core pair (core parity h gets subtiles
s = 2k + h, k = 0..7) so both cores see the identical causal work profile
(512-key chunk counts [1,1,2,2,3,3,4,4]) and a single SPMD program serves all
8 cores; per-core behavior differs only through input data.

The group loop is software-pipelined: group g+1's sim matmuls are issued
ahead of group g's PV/projection so the tensor engine fills group g's
softmax (mask->exp->normalize) latency instead of idling, and stays HAM-warm.
Probability tiles are transposed for the PV matmuls by the DMA XBAR (one
blocked-transpose descriptor per subtile) instead of the tensor engine, and
phase-0 feed DMAs alternate between the sync and activation DGE queues so
transfers run two at a time. The causal diagonal uses exact 256-wide chunks
on even subtiles (the 512 rounding is pure masked-out waste there).

All matmul operands are bf16 (PSUM accumulation in fp32; softmax statistics
in fp32): the elementwise rounding step is 4x fp32r's, far inside the
tolerance, and bf16 enables fast weight load + halves DMA/DVE traffic.
"""

import numpy as np

import concourse.mybir as mybir
import concourse.tile as tile
from concourse import bacc
from concourse.bass_utils import run_bass_kernel_spmd

FP32 = mybir.dt.float32
BF16 = mybir.dt.bfloat16
AF = mybir.ActivationFunctionType
ALU = mybir.AluOpType

B, S, D, NI, NO = 4, 2048, 1024, 1024, 1024
NCORES = 8
P = 128
DC = D // P    # 8 contraction chunks for the projections
IC = NI // P   # 8 inner-dim chunks
NSUB = 8       # local 128-row query subtiles per core
CC = [k // 2 + 1 for k in range(NSUB)]  # 512-key chunks per local subtile
SCALE = float(NI) ** -0.5
NEG = -1.0e9

_CACHED = {}


def _build():
    nc = bacc.Bacc(None, target_bir_lowering=False, debug=False, num_devices=NCORES)

    xT = nc.dram_tensor("xT", [D, S], BF16, kind="ExternalInput").ap()
    # per-d-chunk pack of [Mq rows | xQ low half | xQ high half]: one DMA
    # descriptor feeds a whole phase-0 accumulation step
    mqx_d = nc.dram_tensor("mqx", [D, D + NSUB * P], BF16,
                           kind="ExternalInput").ap()
    xR = nc.dram_tensor("xR", [S, D], BF16, kind="ExternalInput").ap()
    wvo_d = nc.dram_tensor("wvo", [NI, NO], BF16, kind="ExternalInput").ap()
    masks = nc.dram_tensor("masks", [NSUB, P, 512], BF16, kind="ExternalInput").ap()
    bb = nc.dram_tensor("bb", [P, NO], FP32, kind="ExternalInput").ap()
    y = nc.dram_tensor("y", [NSUB * P, NO], BF16, kind="ExternalOutput").ap()

    with tile.TileContext(nc) as tc:
        with (
            tc.tile_pool(name="const", bufs=1) as constp,
            tc.tile_pool(name="xtpool", bufs=2 * IC) as xtp,
            tc.tile_pool(name="qtpool0", bufs=IC) as qtp0,
            tc.tile_pool(name="qtpool1", bufs=IC) as qtp1,
            tc.tile_pool(name="wpool", bufs=DC) as wp,
        ):
            # two separate tile arrays per key-half: a write to the high half
            # must not create a false whole-tile WAR hazard against sim reads
            # of the low half (tile dependencies are tracked per-tile)
            XTa = [xtp.tile([P, S // 2], BF16, name=f"xta{i}", tag="xt")
                   for i in range(IC)]
            XTb = [xtp.tile([P, S // 2], BF16, name=f"xtb{i}", tag="xt")
                   for i in range(IC)]
            # per-qh-half QM^T tiles: keeps group 0's sim free of any
            # dependency on the second half's PSUM->SBUF copies
            QMT = [
                [qtp0.tile([P, 512], BF16, name=f"qt0_{i}", tag="qt0")
                 for i in range(IC)],
                [qtp1.tile([P, 512], BF16, name=f"qt1_{i}", tag="qt1")
                 for i in range(IC)],
            ]

            # ---- Phase 0: QM^T = Mq^T @ xQ for queries 0:512 (groups 0/1) ----
            # d-outer accumulation into 8 concurrently-open PSUM banks: the
            # first matmul only needs mq[0] cols + xq[0] on chip, so the PE
            # starts as soon as the first two transfers land. The second
            # query half (groups 2/3) is computed later, overlapped with the
            # attention pipeline, so its PSUM->SBUF copies never sit ahead of
            # group 0/1's softmax ops in any engine queue.
            mq = []
            xqs = [[], []]
            # alternate the two hardware DGE queues (sync + activation) so
            # the phase-0 feed transfers run two-at-a-time
            for d in range(DC):
                wt = wp.tile([P, D + NSUB * P], BF16, name=f"mqx{d}", tag="w")
                eng = nc.sync if d % 2 == 0 else nc.scalar
                eng.dma_start(out=wt[:], in_=mqx_d[P * d:P * (d + 1), :])
                mq.append(wt)
                xqs[0].append(wt[:, D:D + 512])
                xqs[1].append(wt[:, D + 512:D + 1024])
            # DMA triggers issue serially (~650ns each on the sync engine), so
            # order them by first-use time. x^T comes in column halves: the
            # low half (key chunks 0-1) feeds groups 0/1's sim, the high half
            # isn't read until group 2.
            # masks ahead of the x^T bulk: group 0's first exp needs mask[0]
            # and the per-queue transfers are served in trigger order
            mask_sb = constp.tile([P, NSUB, 512], BF16, name="mask_sb", tag="mask")
            for k in range(4):
                eng = nc.sync if k % 2 == 0 else nc.scalar
                eng.dma_start(out=mask_sb[:, k, :], in_=masks[k])
            for i in range(IC):
                eng = nc.sync if i % 2 == 0 else nc.scalar
                eng.dma_start(out=XTa[i][:],
                              in_=xT[P * i:P * (i + 1), 0:1024])
            # late-use inputs go through the Activation engine's DGE queue:
            # it is idle until the first exp, while the sync queue must stay
            # short so the probability-transpose triggers run on time
            for i in range(IC):
                nc.scalar.dma_start(out=XTb[i][:],
                                    in_=xT[P * i:P * (i + 1), 1024:2048])
            for k in range(4, NSUB):
                nc.sync.dma_start(out=mask_sb[:, k, :], in_=masks[k])

            # rounds of 2 psum banks: round r's PSUM->SBUF casts drain on
            # the DVE while round r+1's matmuls stream, instead of all 8
            # casts bunching into a serial chain after the last matmul
            with tc.tile_pool(name="qacc", bufs=8, space="PSUM") as qacc:
                # HAM warm-up: the PE is otherwise idle for the first ~5us
                # while the feed transfers land, and the free-running
                # activity window would keep phase 0 at the cold 1.2 GHz
                # half-clock. A burst of discarded zero-matmuls (no DMA
                # dependency) gets the un-throttle out of the way for free.
                warm = constp.tile([P, 512], BF16, name="warm", tag="warm")
                nc.vector.memset(warm[:], 0.0)
                wps = qacc.tile([P, 512], FP32, name="warmps", tag="qa")
                for w in range(26):
                    nc.tensor.matmul(
                        wps[:], warm[:, 0:P], warm[:],
                        start=(w == 0), stop=(w == 25),
                    )
                # rounds of 4: per-d consumption (0.86us) stays above the
                # dual-queue arrival rate (~0.7us/chunk), so the feed never
                # stalls the PE mid-phase (a stall here re-throttles HAM),
                # while round A's casts still drain during round B
                for r in range(2):
                    pss = [
                        qacc.tile([P, 512], FP32, name=f"ps_qt0_{i}", tag="qa")
                        for i in range(4 * r, 4 * r + 4)
                    ]
                    for d in range(DC):
                        for j in range(4):
                            nc.tensor.matmul(
                                pss[j][:],
                                mq[d][:, P * (4 * r + j):P * (4 * r + j + 1)],
                                xqs[0][d],
                                start=(d == 0), stop=(d == DC - 1),
                            )
                    for j in range(4):
                        nc.vector.tensor_copy(QMT[0][4 * r + j][:], pss[j][:])

            # ---- attention, 4 pair-groups of 2 subtiles, pipelined ----
            with (
                tc.tile_pool(name="accp", bufs=4, space="PSUM") as accp,
                tc.tile_pool(name="opp", bufs=4, space="PSUM") as opp,
                tc.tile_pool(name="wopool", bufs=DC) as wop,
                tc.tile_pool(name="vfixp", bufs=4) as vfixp,
            ):
                # x rows [0:512) are read by every group: pin them in SBUF
                vfix = []
                for t in range(4):
                    vf = vfixp.tile([P, NI], BF16, name=f"vfix{t}", tag="vfix")
                    nc.sync.dma_start(out=vf[:], in_=xR[P * t:P * (t + 1), :])
                    vfix.append(vf)
                wo = []
                for d in range(DC):
                    t = wop.tile([P, NO], BF16, name=f"wo{d}", tag="wo")
                    nc.sync.dma_start(out=t[:], in_=wvo_d[P * d:P * (d + 1), :])
                    wo.append(t)
                b_sb = constp.tile([P, NO], FP32, name="b_sb", tag="b")
                nc.sync.dma_start(out=b_sb[:], in_=bb[:])
                with (
                    tc.tile_pool(name="ppool", bufs=4) as ppool,
                    tc.tile_pool(name="ptpool", bufs=2) as ptpool,
                    tc.tile_pool(name="otpool", bufs=2 * IC) as otpool,
                    tc.tile_pool(name="vrd", bufs=8) as vrdp,
                    tc.tile_pool(name="ypool", bufs=4) as ypool,
                    tc.tile_pool(name="stp", bufs=24) as stp,
                ):
                    PS = {}
                    PTB = {}
                    OT = {}

                    def qmt1_phase():
                        # second query half (groups 2/3): d-inner accumulation
                        # through the accp ping-pong, overlapped with the
                        # group-0/1 attention pipeline
                        for i in range(IC):
                            ps = accp.tile([P, 512], FP32, name="ps_qt1",
                                           tag="acc")
                            for d in range(DC):
                                nc.tensor.matmul(
                                    ps[:], mq[d][:, P * i:P * (i + 1)],
                                    xqs[1][d],
                                    start=(d == 0), stop=(d == DC - 1),
                                )
                            nc.vector.tensor_copy(QMT[1][i][:], ps[:])

                    def sim_phase(g):
                        L = g + 1
                        nt = 4 * L
                        ptb = ptpool.tile([P, 16, 256], BF16, name=f"ptb{g}",
                                          tag="ptb")
                        for k in (2 * g, 2 * g + 1):
                            # exact causal need: subtile k covers 256*(k+1)
                            # keys. Even k: k/2 full 512 chunks + a 256-wide
                            # diagonal chunk; odd k: (k+1)/2 chunks with the
                            # diagonal mask in the last 512 chunk.
                            even = (k % 2 == 0)
                            ndiag = 256 if even else 512
                            doff = 256 * k if even else 512 * (L - 1)
                            nfull = k // 2 if even else L - 1
                            p_t = ppool.tile([P, 4 * 512], BF16, name=f"p{k}",
                                             tag="p")
                            sums = stp.tile([P, 4], FP32, name=f"sums{k}",
                                            tag="sums")
                            # diagonal chunk first: its mask+exp chain overlaps
                            # the remaining chunks' matmuls
                            for ci, kc in enumerate([-1] + list(range(nfull))):
                                ps = accp.tile([P, 512], FP32, name="ps_sim",
                                               tag="acc")
                                if kc < 0:
                                    off, w = doff, ndiag
                                else:
                                    off, w = 512 * kc, 512
                                psv = ps[:, 0:w]
                                for i in range(IC):
                                    xth = XTa[i] if off < 1024 else XTb[i]
                                    nc.tensor.matmul(
                                        psv,
                                        QMT[k // 4][i][:, P * (k % 4):
                                                       P * (k % 4) + P],
                                        xth[:, off % 1024:off % 1024 + w],
                                        start=(i == 0), stop=(i == IC - 1),
                                    )
                                if kc < 0:
                                    nc.vector.tensor_tensor(
                                        out=psv, in0=psv,
                                        in1=mask_sb[:, k, 0:w],
                                        op=ALU.add,
                                    )
                                nc.scalar.activation(
                                    p_t[:, off:off + w], psv, AF.Exp,
                                    scale=SCALE, accum_out=sums[:, ci:ci + 1],
                                )
                            ssum = stp.tile([P, 1], FP32, name=f"ssum{k}", tag="ss")
                            nc.vector.tensor_reduce(
                                ssum[:], sums[:, :nfull + 1],
                                axis=mybir.AxisListType.X,
                                op=ALU.add,
                            )
                            rsum = stp.tile([P, 1], FP32, name=f"rsum{k}", tag="rs")
                            nc.vector.reciprocal(rsum[:], ssum[:])
                            nc.vector.tensor_scalar_mul(
                                p_t[:, :256 * (k + 1)], p_t[:, :256 * (k + 1)],
                                rsum[:]
                            )
                            PS[k] = p_t
                            # one XBAR DMA per subtile transposes the whole
                            # normalized-probability slab into blocked
                            # [key, t, q] layout; issued here so it overlaps
                            # the previous group's PV/projection work. The
                            # even subtile's last two key blocks aren't
                            # computed (fully masked), so zero them in ptb.
                            half = slice(0, P) if even else slice(P, 256)
                            if even:
                                nc.vector.memset(ptb[:, nt - 2:nt, half], 0.0)
                            nc.sync.dma_start(
                                out=ptb[:, 0:2 * (k + 1), half],
                                in_=p_t[:, 0:256 * (k + 1)],
                                transpose=True,
                            )
                        PTB[g] = ptb

                    def pv_phase(g):
                        L = g + 1
                        k0, k1 = 2 * g, 2 * g + 1
                        ops = [
                            opp.tile([P, 512], FP32, name=f"op{g}_{j}", tag="op")
                            for j in range(4)
                        ]
                        nt = 4 * L
                        ptb = PTB[g]
                        for t in range(nt):
                            if t < 4:
                                v_t = vfix[t]
                            else:
                                v_t = vrdp.tile([P, NI], BF16, name="v_t", tag="v")
                                nc.sync.dma_start(
                                    out=v_t[:], in_=xR[P * t:P * (t + 1), :]
                                )
                            for m in range(IC):
                                # one accumulation group per PSUM bank: start
                                # only on the bank's first matmul (whole-bank
                                # pending-zero makes the sibling column-half's
                                # first write an overwrite), stop on its last
                                nc.tensor.matmul(
                                    ops[m // 2][:, 256 * (m % 2):
                                                256 * (m % 2) + 256],
                                    v_t[:, P * m:P * (m + 1)],
                                    ptb[:, t, :],
                                    start=(t == 0 and m % 2 == 0),
                                    stop=(t == nt - 1 and m % 2 == 1),
                                )

                        oT = []
                        for m in range(IC):
                            ot = otpool.tile([P, 256], BF16, name=f"ot{g}_{m}",
                                             tag="ot")
                            nc.vector.tensor_copy(
                                ot[:],
                                ops[m // 2][:, 256 * (m % 2):256 * (m % 2) + 256]
                            )
                            oT.append(ot)
                        OT[g] = oT

                    def proj_phase(g):
                        # deferred past the next group's sim matmuls so the
                        # serialized attnX^T PSUM->SBUF casts don't stall the
                        # tensor engine between PV and projection
                        k0, k1 = 2 * g, 2 * g + 1
                        oT = OT[g]
                        # ---- output projection for this group's 2 subtiles ----
                        # y psums cycle through the opp pool so accp stays free
                        # for the pipelined sim matmuls; bias-add runs on the
                        # otherwise-idle GpSimd engine
                        for col, k in enumerate((k0, k1)):
                            for oh in range(2):
                                ps = opp.tile([P, 512], FP32, name="ps_y", tag="op")
                                for i in range(IC):
                                    nc.tensor.matmul(
                                        ps[:],
                                        oT[i][:, P * col:P * (col + 1)],
                                        wo[i][:, 512 * oh:512 * (oh + 1)],
                                        start=(i == 0), stop=(i == IC - 1),
                                    )
                                y_sb = ypool.tile([P, 512], BF16, name="y_sb",
                                                  tag="y")
                                nc.vector.tensor_tensor(
                                    out=y_sb[:], in0=ps[:],
                                    in1=b_sb[:, 512 * oh:512 * (oh + 1)],
                                    op=ALU.add,
                                )
                                nc.sync.dma_start(
                                    out=y[P * k:P * (k + 1),
                                          512 * oh:512 * (oh + 1)],
                                    in_=y_sb[:],
                                )

                    sim_phase(0)
                    sim_phase(1)
                    qmt1_phase()
                    pv_phase(0)
                    sim_phase(2)
                    proj_phase(0)
                    pv_phase(1)
                    sim_phase(3)
                    proj_phase(1)
                    pv_phase(2)
                    proj_phase(2)
                    pv_phase(3)
                    proj_phase(3)

    nc.compile()
    return nc


def _prep_inputs(x, w_qkv, w_out, b_out):
    import ml_dtypes
    BF = ml_dtypes.bfloat16
    x = np.asarray(x, dtype=np.float32)
    w_qkv = np.asarray(w_qkv, dtype=np.float32)
    w_out = np.asarray(w_out, dtype=np.float32)
    b_out = np.asarray(b_out, dtype=np.float32)

    wq = w_qkv[:, 0 * NI:1 * NI]
    wk = w_qkv[:, 1 * NI:2 * NI]
    wv = w_qkv[:, 2 * NI:3 * NI]
    mq = np.ascontiguousarray((wq @ wk.T).astype(BF))
    wvo = np.ascontiguousarray((wv @ w_out).astype(BF))
    b_bcast = np.ascontiguousarray(np.broadcast_to(b_out[None, :], (P, NO)))

    xbf = [x[b].astype(BF) for b in range(B)]
    xTs = [np.ascontiguousarray(xb.T) for xb in xbf]

    in_maps = []
    for c in range(NCORES):
        b, h = c // 2, c % 2
        subs = [2 * k + h for k in range(NSUB)]
        xQc = np.concatenate(
            [xTs[b][:, P * s:P * (s + 1)] for s in subs], axis=1
        )
        mqx = np.ascontiguousarray(np.concatenate([mq, xQc], axis=1))
        m = np.empty((NSUB, P, 512), dtype=BF)
        cpos = np.arange(512)[None, :]
        prow = np.arange(P)[:, None]
        for k in range(NSUB):
            if k % 2 == 0:
                # 256-wide diagonal chunk at key offset 256k
                off = P * subs[k] - 256 * k
                row = np.where(cpos <= off + prow, 0.0, NEG)
                row[:, 256:] = NEG
                m[k] = row
            else:
                off = P * subs[k] - 512 * (CC[k] - 1)
                m[k] = np.where(cpos <= off + prow, 0.0, NEG)
        in_maps.append({
            "xT": xTs[b], "xR": xbf[b],
            "mqx": mqx, "wvo": wvo,
            "masks": m, "bb": b_bcast,
        })
    return in_maps


def _run(x, w_qkv, w_out, b_out, trace=False, **kw):
    if "nc" not in _CACHED:
        _CACHED["nc"] = _build()
    nc = _CACHED["nc"]
    in_maps = _prep_inputs(x, w_qkv, w_out, b_out)
    res = run_bass_kernel_spmd(nc, in_maps, list(range(NCORES)), trace=trace, **kw)
    out = np.empty((B, S, NO), dtype=np.float32)
    for c in range(NCORES):
        b, h = c // 2, c % 2
        yc = np.asarray(res.results[c]["y"], dtype=np.float32)
        for k in range(NSUB):
            s = 2 * k + h
            out[b, P * s:P * (s + 1), :] = yc[P * k:P * (k + 1), :]
    return out, res


def kernel(x, w_qkv, w_out, b_out):
    out, _ = _run(x, w_qkv, w_out, b_out, trace=False)
    return out

